# revision 1
# baseline (speedup 1.0000x reference)
"""Multi-head causal attention block on 8 Trainium2 NeuronCores.

Sharding: tensor-parallel over heads (4 groups of 4 heads) x data-parallel
over batch (2). Core c -> (batch b=c//4, head-group g=c%4). Each core
computes q/k/v projections for its head group, causal attention for its 4
heads, and a partial output projection; the host sums the 4 partials per
batch. All layout transposes are done host-side so the device does none.

Self-contained: hardcodes shapes for the 2x2048x2048, 16-head problem.
"""

import os
from contextlib import ExitStack

import numpy as np

import concourse.bass as bass
import concourse.tile as tile
from concourse import bacc, mybir
from concourse.bass import ds, ts
from concourse.bass_utils import run_bass_kernel_spmd

F32 = mybir.dt.float32
F32R = mybir.dt.float32r
ACTF = mybir.ActivationFunctionType

# Full-problem dims
BATCH = 2
SEQ = 2048
D_MODEL = 2048
NUM_HEADS = 16
HEAD_DIM = 128
N_CORES = 8
N_GROUPS = 4  # head-groups (tensor parallel)
DG = D_MODEL // N_GROUPS  # 512 = 4 heads per group
SCALE = 1.0 / float(np.sqrt(HEAD_DIM))

QB = 512  # q-block width in attention
KT = 128  # k-tile width (partition dim)

USE_F32R = os.environ.get("KERNEL_F32", "0") != "1"
MMDT = F32R if USE_F32R else F32


def _r(ap):
    """View a float32 DRAM AP as the matmul dtype for DMA into MMDT tiles."""
    return ap.bitcast(F32R) if USE_F32R else ap


def _mha_body(ctx, tc, aps, S, D, DGl):
    """Per-core kernel body.

    aps: dict of DRAM APs: xt [D,S], wqt/wkt/wvt [D,DGl], wot [DGl,D],
      bq/bk [128, DGl//128], bv [128, DGl], bo [128, D], masks [4,128,QB],
      ones [128,1], out [S,D].

    k^T and v stay resident in SBUF (written directly by the projection
    drains); only q^T round-trips through DRAM.
    """
    nc = tc.nc
    n_kd = D // 128  # contraction tiles over d_model
    n_sq = S // QB  # 512-wide attention q-blocks
    n_sk = S // KT  # 128-wide seq tiles
    n_dg = DGl // 128  # head tiles per group
    QB1 = 256  # phase-1 seq-slice width
    n_ns = S // QB1

    xt, wqt, wkt, wvt, wot = aps["xt"], aps["wqt"], aps["wkt"], aps["wvt"], aps["wot"]
    out = aps["out"]

    # DRAM scratch for v [S, DGl] (q^T and k^T stay resident in SBUF)
    dram = ctx.enter_context(tc.tile_pool(name="dram", bufs=1, space="DRAM"))
    v_d = dram.tile([S, DGl], F32, name="v_d")

    consts = ctx.enter_context(tc.tile_pool(name="consts", bufs=1))
    # dummy activation first: forces the ACT function-table DMA to queue
    # ahead of the bulk input loads (else every early PSUM drain stalls)
    warm = consts.tile([128, 1], F32, name="act_warm")
    nc.vector.memset(warm[:], 0.0)
    nc.scalar.activation(warm[:], warm[:], ACTF.Identity, bias=warm[:, 0:1])
    ones_sb = consts.tile([128, 1], MMDT, name="ones_sb")
    bq_sb = consts.tile([128, n_dg], F32, name="bq_sb")
    bk_sb = consts.tile([128, n_dg], F32, name="bk_sb")
    bv_sb = consts.tile([128, DGl], F32, name="bv_sb")
    masks_sb = consts.tile([128, 4 * QB], F32, name="masks_sb")

    # resident q^T / k^T per head: [p, s] = q^T/k^T[h*128+p, s]
    kv_pool = ctx.enter_context(tc.tile_pool(name="kv_res", bufs=1))
    kt_res = [
        kv_pool.tile([128, S], MMDT, tag=f"ktr{h}", name=f"kt_res{h}")
        for h in range(n_dg)
    ]
    qt_res = [
        kv_pool.tile([128, S], MMDT, tag=f"qtr{h}", name=f"qt_res{h}")
        for h in range(n_dg)
    ]

    # ---------------- Phase 1: q/k/v projections ----------------
    with (
        tc.tile_pool(name="wqkv", bufs=1) as wpool,
        tc.tile_pool(name="xt_pool", bufs=2) as xpool,
        tc.tile_pool(name="p1_stage", bufs=2) as stage,
        tc.tile_pool(name="p1_psum", bufs=4, space="PSUM") as psum1,
    ):
        # weights resident: w*_sb[p, k*DGl + f] = w*t[k*128+p, f]
        w_sbs = {
            wname: wpool.tile([128, n_kd * DGl], MMDT, name=f"{wname}_sb")
            for wname in ("wq", "wk", "wv")
        }

        def load_w(wname, wap):
            nc.sync.dma_start(
                w_sbs[wname][:].rearrange("p (k f) -> p k f", k=n_kd),
                _r(wap).rearrange("(k p) f -> p k f", p=128),
            )

        def load_w_mblock(wname, wap, m):
            nc.sync.dma_start(
                w_sbs[wname][:].rearrange(
                    "p (k g j) -> p k g j", k=n_kd, j=128
                )[:, :, m, :],
                _r(wap).rearrange("(k p) (g j) -> p k g j", p=128, j=128)[
                    :, :, m, :
                ],
            )

        def load_xt(ns):
            # two k-half DMAs: the slice's first k-accumulations can start
            # as soon as the first half lands
            t = xpool.tile([128, n_kd * QB1], MMDT, tag="xt", name="xt_sb")
            half = n_kd // 2
            for hlf in range(2):
                nc.sync.dma_start(
                    t[:, ds(hlf * half * QB1, half * QB1)].rearrange(
                        "p (k f) -> p k f", k=half
                    ),
                    _r(
                        xt[ds(hlf * half * 128, half * 128), ts(ns, QB1)]
                    ).rearrange("(k p) f -> p k f", p=128),
                )
            return t

        nc.sync.dma_start(ones_sb[:], _r(aps["ones"]))
        nc.sync.dma_start(bq_sb[:], aps["bq"])
        nc.sync.dma_start(bk_sb[:], aps["bk"])
        nc.sync.dma_start(bv_sb[:], aps["bv"])
        # k^T first: PE can start on wk+x0 while wq/wv still stream in
        load_w_mblock("wk", wkt, 0)
        g0 = load_xt(0)
        for m in range(1, n_dg):
            load_w_mblock("wk", wkt, m)
        g1 = load_xt(1)
        for m in range(n_dg):
            load_w_mblock("wq", wqt, m)
        load_w("wv", wvt)
        nc.sync.dma_start(
            masks_sb[:].rearrange("p (i f) -> p i f", i=4),
            aps["masks"].rearrange("i p f -> p i f"),
        )

        def do_proj_t(res, wname, b_sb, ns, xt_sb):
            # q^T/k^T [m hd-dims 128, QB1 seq] drains into resident tiles
            for m in range(n_dg):
                ps = psum1.tile([128, QB1], F32, tag="ps", name="ps_qk")
                for k in range(n_kd):
                    nc.tensor.matmul(
                        ps[:],
                        lhsT=w_sbs[wname][:, ds(k * DGl + m * 128, 128)],
                        rhs=xt_sb[:, ts(k, QB1)],
                        start=(k == 0),
                        stop=(k == n_kd - 1),
                    )
                nc.scalar.activation(
                    res[m][:, ts(ns, QB1)],
                    ps[:],
                    ACTF.Identity,
                    bias=b_sb[:, ds(m, 1)],
                )

        def do_v(ns, xt_sb):
            for msub in range(QB1 // 128):
                ps = psum1.tile([128, DGl], F32, tag="ps", name="ps_v")
                for k in range(n_kd):
                    nc.tensor.matmul(
                        ps[:],
                        lhsT=xt_sb[:, ds(k * QB1 + msub * 128, 128)],
                        rhs=w_sbs["wv"][:, ts(k, DGl)],
                        start=(k == 0),
                        stop=(k == n_kd - 1),
                    )
                st = stage.tile([128, DGl], F32, tag="v_st", name="v_st")
                nc.vector.tensor_add(st[:], ps[:], bv_sb[:])
                nc.sync.dma_start(
                    v_d[ds(ns * QB1 + msub * 128, 128), :], st[:]
                )

        # head group: k^T for slices 0-1 (no DMA drains), then q^T, then v
        for ns, g in ((0, g0), (1, g1)):
            do_proj_t(kt_res, "wk", bk_sb, ns, g)
        for ns, g in ((0, g0), (1, g1)):
            do_proj_t(qt_res, "wq", bq_sb, ns, g)
        for ns, g in ((0, g0), (1, g1)):
            do_v(ns, g)
        nxt = load_xt(2) if n_ns > 2 else None
        for ns in range(2, n_ns):
            xt_sb = nxt
            nxt = load_xt(ns + 1) if ns + 1 < n_ns else None
            do_proj_t(kt_res, "wk", bk_sb, ns, xt_sb)
            do_proj_t(qt_res, "wq", bq_sb, ns, xt_sb)
            do_v(ns, xt_sb)

    # ---------------- Phase 2: causal attention ----------------
    # ctx^T per head stays resident in SBUF for phase 3
    ctx_pool = ctx.enter_context(tc.tile_pool(name="ctx_pool", bufs=1))
    ctx_sbs = [
        ctx_pool.tile([128, S], MMDT, tag=f"ctx{h}", name=f"ctx_sb{h}")
        for h in range(n_dg)
    ]

    # wo stays resident; loaded mid-phase-2 so phase 3 starts hot
    wopool = ctx.enter_context(tc.tile_pool(name="wo_pool", bufs=1))
    wo_sb = wopool.tile([128, n_dg * D], MMDT, name="wo_sb")

    # phase-2/3-only constants live after phase-1 pools are freed
    p2consts = ctx.enter_context(tc.tile_pool(name="p2consts", bufs=1))
    bo_sb = p2consts.tile([128, D], F32, name="bo_sb")
    nc.sync.dma_start(bo_sb[:], aps["bo"])

    with (
        tc.tile_pool(name="v_pool", bufs=3) as vpool,
        tc.tile_pool(name="exp_pool", bufs=8) as epool,
        tc.tile_pool(name="lrec_pool", bufs=3) as lpool,
        tc.tile_pool(name="bc_pool", bufs=3) as bcpool,
        tc.tile_pool(name="ps_s", bufs=5, space="PSUM") as ps_s_pool,
        tc.tile_pool(name="ps_c", bufs=2, space="PSUM") as ps_c_pool,
        tc.tile_pool(name="ps_l", bufs=1, space="PSUM") as ps_l_pool,
    ):
        for h in range(n_dg):
            # v_sb[p, t*128+j] = v[t*128+p, h*128+j]; quarter DMAs so the
            # first q-blocks' PV can start before the whole head lands
            v_sb = vpool.tile([128, n_sk * 128], MMDT, tag="v", name="v_sb")
            nq = max(1, S // 512)
            for vq in range(nq):
                nc.sync.dma_start(
                    v_sb[:, ds(vq * 512, 512)].rearrange(
                        "p (t j) -> p t j", j=128
                    ),
                    _r(v_d[ds(vq * 512, 512), ts(h, 128)]).rearrange(
                        "(t p) j -> p t j", p=128
                    ),
                )
            if h == 1:
                # wo_sb[p, k*D + f] = wot[k*128+p, f] (phase-3 prefetch)
                nc.sync.dma_start(
                    wo_sb[:].rearrange("p (k f) -> p k f", k=n_dg),
                    _r(wot).rearrange("(k p) f -> p k f", p=128),
                )
            for qb in range(n_sq):
                n_kt = (qb + 1) * (QB // KT)  # causal: only k-tiles <= q
                ps_c = ps_c_pool.tile([128, QB], F32, tag="c", name="ps_c")
                ps_l = ps_l_pool.tile([1, QB], F32, tag="l", name="ps_l")
                diag0 = n_kt - (QB // KT)
                for kt in range(n_kt):
                    off = kt - diag0
                    # causal column restriction: diagonal tile off needs
                    # only cols >= off*128; keep moving dim >= 256 for
                    # full-rate f32r (so off=3 starts at 256, masked).
                    sc = 0 if off < 1 else (128 if off == 1 else 256)
                    w = QB - sc
                    ps_sc = ps_s_pool.tile([128, QB], F32, tag="s", name="ps_sc")
                    nc.tensor.matmul(
                        ps_sc[:, ds(sc, w)],
                        lhsT=kt_res[h][:, ts(kt, 128)],
                        rhs=qt_res[h][:, ds(qb * QB + sc, w)],
                        start=True,
                        stop=True,
                    )
                    if off >= 0:
                        # only the triangular block (plus, for off=3, the
                        # fully-invalid 128 cols kept for moving-dim>=256)
                        # needs masking; columns right of it are all-valid
                        msc = off * 128 if off < 3 else 256
                        mw = 128 if off < 3 else 256
                        nc.vector.tensor_add(
                            ps_sc[:, ds(msc, mw)],
                            ps_sc[:, ds(msc, mw)],
                            masks_sb[:, ds(off * QB + msc, mw)],
                        )
                    ex = epool.tile([128, QB], MMDT, tag="e", name="ex")
                    nc.scalar.activation(
                        ex[:, ds(sc, w)], ps_sc[:, ds(sc, w)], ACTF.Exp, scale=SCALE
                    )
                    nc.tensor.matmul(
                        ps_c[:, ds(sc, w)],
                        lhsT=v_sb[:, ts(kt, 128)],
                        rhs=ex[:, ds(sc, w)],
                        start=(kt == 0),
                        stop=(kt == n_kt - 1),
                        skip_group_check=True,
                    )
                    nc.tensor.matmul(
                        ps_l[:, ds(sc, w)],
                        lhsT=ones_sb[:],
                        rhs=ex[:, ds(sc, w)],
                        start=(kt == 0),
                        stop=(kt == n_kt - 1),
                        skip_group_check=True,
                    )
                rec = lpool.tile([1, QB], F32, tag="r", name="rec")
                nc.vector.reciprocal(rec[:], ps_l[:])
                bc = bcpool.tile([128, QB], F32, tag="bc", name="bc")
                nc.gpsimd.partition_broadcast(bc[:], rec[:])
                nc.vector.tensor_mul(
                    ctx_sbs[h][:, ts(qb, QB)], ps_c[:], bc[:]
                )

    # ---------------- Phase 3: output projection ----------------
    with (
        tc.tile_pool(name="o_stage", bufs=4) as ostage,
        tc.tile_pool(name="p3_psum", bufs=4, space="PSUM") as psum3,
    ):
        for m in range(n_sk):
            for n in range(D // QB):
                ps = psum3.tile([128, QB], F32, tag="o", name="ps_p3")
                for k in range(n_dg):
                    nc.tensor.matmul(
                        ps[:],
                        lhsT=ctx_sbs[k][:, ts(m, 128)],
                        rhs=wo_sb[:, ds(k * D + n * QB, QB)],
                        start=(k == 0),
                        stop=(k == n_dg - 1),
                    )
                ot = ostage.tile([128, QB], F32, tag="ot", name="ot")
                nc.vector.tensor_add(ot[:], ps[:], bo_sb[:, ts(n, QB)])
                nc.sync.dma_start(out[ts(m, 128), ts(n, QB)], ot[:])


def build_program(S=SEQ, D=D_MODEL, DGl=DG, enable_asserts=False):
    nc = bacc.Bacc(
        "TRN2",
        target_bir_lowering=False,
        debug=False,
        enable_asserts=enable_asserts,
        num_devices=N_CORES,
    )
    aps = {
        "xt": nc.dram_tensor("xt", [D, S], F32, kind="ExternalInput").ap(),
        "wqt": nc.dram_tensor("wqt", [D, DGl], F32, kind="ExternalInput").ap(),
        "wkt": nc.dram_tensor("wkt", [D, DGl], F32, kind="ExternalInput").ap(),
        "wvt": nc.dram_tensor("wvt", [D, DGl], F32, kind="ExternalInput").ap(),
        "wot": nc.dram_tensor("wot", [DGl, D], F32, kind="ExternalInput").ap(),
        "bq": nc.dram_tensor("bq", [128, DGl // 128], F32, kind="ExternalInput").ap(),
        "bk": nc.dram_tensor("bk", [128, DGl // 128], F32, kind="ExternalInput").ap(),
        "bv": nc.dram_tensor("bv", [128, DGl], F32, kind="ExternalInput").ap(),
        "bo": nc.dram_tensor("bo", [128, D], F32, kind="ExternalInput").ap(),
        "masks": nc.dram_tensor("masks", [4, 128, QB], F32, kind="ExternalInput").ap(),
        "ones": nc.dram_tensor("ones", [128, 1], F32, kind="ExternalInput").ap(),
        "out": nc.dram_tensor("out", [S, D], F32, kind="ExternalOutput").ap(),
    }
    with tile.TileContext(nc) as tc:
        with ExitStack() as ctx:
            _mha_body(ctx, tc, aps, S, D, DGl)
    nc.compile()
    return nc


def make_masks():
    """Additive causal masks: 0 where k<=q, -1e30 where masked."""
    i = np.arange(4)[:, None, None]
    p = np.arange(128)[None, :, None]
    f = np.arange(QB)[None, None, :]
    keep = (i * 128 + p) <= f
    return np.where(keep, 0.0, -1e30).astype(np.float32)


def shard_inputs(x, wq, bq, wk, bk, wv, bv, wo, bo):
    """Build the 8 per-core input maps (host-side layout prep)."""
    masks = make_masks()
    xts = [np.ascontiguousarray(np.asarray(x[b], np.float32).T) for b in range(BATCH)]
    bo_bc = np.ascontiguousarray(
        np.broadcast_to(np.asarray(bo, np.float32), (128, D_MODEL))
    )
    bo_zero = np.zeros((128, D_MODEL), np.float32)
    in_maps = []
    for c in range(N_CORES):
        b, g = divmod(c, N_GROUPS)
        sl = slice(g * DG, (g + 1) * DG)
        in_maps.append(
            {
                "xt": xts[b],
                "wqt": np.ascontiguousarray(np.asarray(wq, np.float32)[sl].T),
                "wkt": np.ascontiguousarray(np.asarray(wk, np.float32)[sl].T),
                "wvt": np.ascontiguousarray(np.asarray(wv, np.float32)[sl].T),
                "wot": np.ascontiguousarray(np.asarray(wo, np.float32)[:, sl].T),
                "bq": np.ascontiguousarray(
                    np.asarray(bq, np.float32)[sl].reshape(-1, 128).T
                ),
                "bk": np.ascontiguousarray(
                    np.asarray(bk, np.float32)[sl].reshape(-1, 128).T
                ),
                "bv": np.ascontiguousarray(
                    np.broadcast_to(np.asarray(bv, np.float32)[sl], (128, DG))
                ),
                "bo": bo_bc if g == 0 else bo_zero,
                "masks": masks,
                "ones": np.ones((128, 1), np.float32),
            }
        )
    return in_maps


_NC_CACHE = {}


def get_program():
    if "nc" not in _NC_CACHE:
        _NC_CACHE["nc"] = build_program()
    return _NC_CACHE["nc"]


def run_sharded(inputs, trace=False):
    nc = get_program()
    in_maps = shard_inputs(**inputs)
    res = run_bass_kernel_spmd(nc, in_maps, list(range(N_CORES)), trace=trace)
    full = np.empty((BATCH, SEQ, D_MODEL), np.float32)
    for b in range(BATCH):
        acc = res.results[b * N_GROUPS]["out"].copy()
        for g in range(1, N_GROUPS):
            acc += res.results[b * N_GROUPS + g]["out"]
        full[b] = acc
    return full, res


def kernel(**inputs):
    out, _ = run_sharded(inputs, trace=False)
    return out



# revision 24
# speedup vs baseline: 1.1316x; 1.1316x over previous
"""Multi-head causal attention block on 8 Trainium2 NeuronCores.

Sharding: tensor-parallel over heads (4 groups of 4 heads) x data-parallel
over batch (2). Core c -> (batch b=c//4, head-group g=c%4). Each core
computes q/k/v projections for its head group, causal attention for its 4
heads, and a partial output projection; the host sums the 4 partials per
batch and adds (bo + bv @ wo^T) once (softmax rows sum to 1, so the v-bias
contributes exactly bv @ wo^T).

All matmul operands are bf16. The kernel is a single pipelined pass:
window s runs projection slice s, attention q-block s-1, and output
projection for q-block s-2 interleaved at ~850ns granularity so the PE
stream never blocks on Activation-engine exp. The softmax denominator is
built from DVE wide-folds of the exp tiles plus short accumulating
ones-matmuls (cheap on PE), not a full ones-matmul per k-tile.

Self-contained: hardcodes shapes for the 2x2048x2048, 16-head problem.
"""

from contextlib import ExitStack

import numpy as np

import concourse.bass as bass
import concourse.tile as tile
from concourse import bacc, mybir
from concourse.bass import ds, ts
from concourse.bass_utils import run_bass_kernel_spmd

F32 = mybir.dt.float32
BF16 = mybir.dt.bfloat16
ACTF = mybir.ActivationFunctionType

# Full-problem dims
BATCH = 2
SEQ = 2048
D_MODEL = 2048
NUM_HEADS = 16
HEAD_DIM = 128
N_CORES = 8
N_GROUPS = 4  # head-groups (tensor parallel)
DG = D_MODEL // N_GROUPS  # 512 = 4 heads per group
SCALE = 1.0 / float(np.sqrt(HEAD_DIM))

SL = 512  # projection slice width (seq) == attention q-block width
QB = 512
KT = 128
N_SL = SEQ // SL  # 4
N_KD = D_MODEL // 128  # 16 contraction tiles
N_DG = DG // 128  # 4 heads per group


def _interleave(primary, filler):
    """Merge unit lists: spread primary units evenly among filler units.

    Each unit is a zero-arg callable. Emits all units exactly once.
    """
    np_, nf = len(primary), len(filler)
    if np_ == 0:
        for u in filler:
            u()
        return
    if nf == 0:
        for u in primary:
            u()
        return
    fi = 0
    for i, u in enumerate(primary):
        u()
        # after primary unit i, emit filler up to proportional position
        target = (i + 1) * nf // np_
        while fi < target:
            filler[fi]()
            fi += 1
    while fi < nf:
        filler[fi]()
        fi += 1


def _mha_body(ctx, tc, aps, S, D, DGl):
    nc = tc.nc
    n_sl = S // SL
    xt, out = aps["xt"], aps["out"]
    wts = {"wk": aps["wkt"], "wq": aps["wqt"], "wv": aps["wvt"]}

    # ---------------- persistent SBUF tiles ----------------
    consts = ctx.enter_context(tc.tile_pool(name="consts", bufs=1))
    warm = consts.tile([128, 1], F32, name="act_warm")
    nc.vector.memset(warm[:], 0.0)
    nc.scalar.activation(warm[:], warm[:], ACTF.Identity, bias=warm[:, 0:1])
    # merged small constants: one f32 DMA (bq|bk) + one bf16 DMA (ones|mask)
    bqk_sb = consts.tile([128, 2 * N_DG], F32, name="bqk_sb")
    om_sb = consts.tile([128, 129], BF16, name="om_sb")

    def ones_ap():
        return om_sb[:, ds(0, 1)]

    def mask_ap():
        return om_sb[:, ds(1, 128)]

    wpool = ctx.enter_context(tc.tile_pool(name="wpool", bufs=1))
    w_sbs = {
        wname: wpool.tile([128, N_KD * DGl], BF16, name=f"{wname}_sb")
        for wname in ("wk", "wq", "wv")
    }
    wo_sb = wpool.tile([128, N_DG * D], BF16, name="wo_sb")

    kt_pool = ctx.enter_context(tc.tile_pool(name="kt_pool", bufs=1))
    kt_res = [kt_pool.tile([128, S], BF16, name=f"kt{h}") for h in range(N_DG)]
    v_res = [kt_pool.tile([128, DGl], BF16, name=f"v{t}") for t in range(S // 128)]
    ctx_sbs = [kt_pool.tile([128, S], BF16, name=f"ctx{h}") for h in range(N_DG)]

    xpool = ctx.enter_context(tc.tile_pool(name="xpool", bufs=2))
    qt_pool = ctx.enter_context(tc.tile_pool(name="qt_pool", bufs=2))
    lrec_pool = ctx.enter_context(tc.tile_pool(name="lrec_pool", bufs=2))
    bc_pool = ctx.enter_context(tc.tile_pool(name="bc_pool", bufs=2))
    ostage = ctx.enter_context(tc.tile_pool(name="ostage", bufs=2))

    mm_ps = ctx.enter_context(tc.tile_pool(name="mm_ps", bufs=2, space="PSUM"))
    sc_ps = ctx.enter_context(tc.tile_pool(name="sc_ps", bufs=2, space="PSUM"))
    c_ps = ctx.enter_context(tc.tile_pool(name="c_ps", bufs=2, space="PSUM"))

    # ---------------- DMA helpers ----------------
    def load_w_mblock(wname, m0, nm):
        nc.sync.dma_start(
            w_sbs[wname][:].rearrange("p (k g j) -> p k g j", k=N_KD, j=128)[
                :, :, ds(m0, nm), :
            ],
            wts[wname].rearrange("(k p) (g j) -> p k g j", p=128, j=128)[
                :, :, ds(m0, nm), :
            ],
        )

    def load_w_khalf(wname, hlf):
        half = N_KD // 2
        nc.sync.dma_start(
            w_sbs[wname][:, ds(hlf * half * DGl, half * DGl)].rearrange(
                "p (k f) -> p k f", k=half
            ),
            wts[wname].rearrange("(k p) f -> p k f", p=128)[
                :, ds(hlf * half, half), :
            ],
        )

    def load_wo():
        nc.sync.dma_start(
            wo_sb[:].rearrange("p (k f) -> p k f", k=N_DG),
            aps["wot"].rearrange("(k p) f -> p k f", p=128),
        )

    def load_xt_half(s, hlf, t):
        # t is a dedicated half-slice tile [128, 8*SL]
        half = N_KD // 2
        nc.sync.dma_start(
            t[:].rearrange("p (k f) -> p k f", k=half),
            xt[ds(hlf * half * 128, half * 128), ts(s, SL)].rearrange(
                "(k p) f -> p k f", p=128
            ),
        )

    # ---------------- projection units for slice s ----------------
    # 12 psum tiles per slice (4 k-m, 4 v-sub, 4 q-m), each 16 matmuls
    # emitted as 4 chunks of 4, plus a drain. x slice is two half tiles
    # (xa: k 0-7, xb: k 8-15) so the first chunks only depend on xa's DMA.
    def proj_units(s, xa, xb, qt_sb):
        units = []
        half = N_KD // 2

        def xs(k, off=0, w=SL):
            t = xa if k < half else xb
            return t[:, ds((k % half) * SL + off, w)]

        def qk_tile(wname, m, res_ap, bias_off):
            ps = mm_ps.tile([128, SL], F32, tag="mm", name="ps_qk")

            def chunk(k0, k1):
                def u():
                    for k in range(k0, k1):
                        nc.tensor.matmul(
                            ps[:],
                            lhsT=w_sbs[wname][:, ds(k * DGl + m * 128, 128)],
                            rhs=xs(k),
                            start=(k == 0),
                            stop=(k == N_KD - 1),
                        )
                return u

            for c in range(4):
                units.append(chunk(c * 4, c * 4 + 4))

            def drain():
                nc.scalar.activation(
                    res_ap, ps[:], ACTF.Identity, bias=bqk_sb[:, ds(bias_off + m, 1)]
                )

            units.append(drain)

        def v_tile(msub):
            ps = mm_ps.tile([128, DGl], F32, tag="mm", name="ps_v")

            def chunk(k0, k1):
                def u():
                    for k in range(k0, k1):
                        nc.tensor.matmul(
                            ps[:],
                            lhsT=xs(k, msub * 128, 128),
                            rhs=w_sbs["wv"][:, ts(k, DGl)],
                            start=(k == 0),
                            stop=(k == N_KD - 1),
                        )
                return u

            for c in range(4):
                units.append(chunk(c * 4, c * 4 + 4))

            def drain():
                nc.scalar.copy(v_res[s * 4 + msub][:], ps[:])

            units.append(drain)

        for m in range(N_DG):
            qk_tile("wk", m, kt_res[m][:, ts(s, SL)], N_DG)
        for msub in range(4):
            v_tile(msub)
        for m in range(N_DG):
            qk_tile("wq", m, qt_sb[:, ts(m, SL)], 0)
        return units

    # ---------------- attention units for q-block qb ----------------
    def attn_units(qb, qt_sb, ex_pool):
        units = []
        n_kt = (qb + 1) * 4
        diag0 = qb * 4
        n_pair = n_kt // 2

        for h in range(N_DG):
            ex = ex_pool.tile([128, n_kt * 512], BF16, tag="ex", name="ex")
            ps_c = c_ps.tile([128, QB], F32, tag="c", name="ps_c")
            state = {}

            def sc_of(kt, diag0=diag0):
                off = kt - diag0
                return 0 if off < 0 else off * 128

            def pair_unit(p, h=h, ex=ex, ps_c=ps_c, state=state):
                def u():
                    kts = (2 * p, 2 * p + 1)
                    is_diag = kts[0] >= diag0
                    ps_s = sc_ps.tile([128, 1024], F32, tag="s", name="ps_s")
                    for i, kt in enumerate(kts):
                        sc = sc_of(kt)
                        nc.tensor.matmul(
                            ps_s[:, ds(i * 512 + sc, 512 - sc)],
                            lhsT=kt_res[h][:, ts(kt, 128)],
                            rhs=qt_sb[:, ds(h * SL + sc, 512 - sc)],
                            start=True,
                            stop=True,
                        )
                    # PVs of previous pair (software pipeline)
                    if "prev" in state:
                        pp = state["prev"]
                        for kt in (2 * pp, 2 * pp + 1):
                            sc = sc_of(kt)
                            nc.tensor.matmul(
                                ps_c[:, ds(sc, 512 - sc)],
                                lhsT=v_res[kt][:, ts(h, 128)],
                                rhs=ex[:, ds(kt * 512 + sc, 512 - sc)],
                                start=(kt == 0),
                                stop=(kt == n_kt - 1),
                                skip_group_check=True,
                            )
                    if not is_diag:
                        # merged exp over both (fully written) halves
                        nc.scalar.activation(
                            ex[:, ds(kts[0] * 512, 1024)],
                            ps_s[:],
                            ACTF.Exp,
                            scale=SCALE,
                        )
                    else:
                        # diag tiles: individually trimmed exps (the psum
                        # region left of each sc is unwritten), then zero
                        # the triangular band via mask multiply
                        for i, kt in enumerate(kts):
                            sc = sc_of(kt)
                            nc.scalar.activation(
                                ex[:, ds(kt * 512 + sc, 512 - sc)],
                                ps_s[:, ds(i * 512 + sc, 512 - sc)],
                                ACTF.Exp,
                                scale=SCALE,
                            )
                        for i, kt in enumerate(kts):
                            off = kt - diag0
                            band = kt * 512 + off * 128
                            nc.vector.tensor_mul(
                                ex[:, ds(band, 128)],
                                ex[:, ds(band, 128)],
                                mask_ap(),
                            )
                    state["prev"] = p
                return u

            for p in range(n_pair):
                units.append(pair_unit(p))

            def tail_a(h=h, ex=ex, ps_c=ps_c, state=state):
                def u():
                    # last pair's PVs
                    pp = state["prev"]
                    for kt in (2 * pp, 2 * pp + 1):
                        sc = sc_of(kt)
                        nc.tensor.matmul(
                            ps_c[:, ds(sc, 512 - sc)],
                            lhsT=v_res[kt][:, ts(h, 128)],
                            rhs=ex[:, ds(kt * 512 + sc, 512 - sc)],
                            start=(kt == 0),
                            stop=(kt == n_kt - 1),
                            skip_group_check=True,
                        )
    # fold full sections down to <= 4 on DVE (bf16 2x mode) in
                    # place into ex sections 0..3, then fold the trimmed
                    # diagonal sections on top. All PVs for this head are
                    # done, so in-place ex edits are safe.
                    n_full = diag0
                    with nc.allow_low_precision(reason="colsum fold, <=3 roundings"):
                        if n_full == 0:
                            # qb0: fold trimmed diags onto section 0
                            for off in range(1, 4):
                                sc = off * 128
                                nc.vector.tensor_add(
                                    ex[:, ds(sc, 512 - sc)],
                                    ex[:, ds(sc, 512 - sc)],
                                    ex[:, ds(off * 512 + sc, 512 - sc)],
                                )
                            state["nsec"] = 1
                            return
                        if n_full >= 8:
                            nc.vector.tensor_add(
                                ex[:, ds(0, 2048)],
                                ex[:, ds(0, 2048)],
                                ex[:, ds(2048, 2048)],
                            )
                        if n_full == 12:
                            nc.vector.tensor_add(
                                ex[:, ds(0, 2048)],
                                ex[:, ds(0, 2048)],
                                ex[:, ds(4096, 2048)],
                            )
                        for off in range(4):
                            sc = off * 128
                            nc.vector.tensor_add(
                                ex[:, ds(off * 512 + sc, 512 - sc)],
                                ex[:, ds(off * 512 + sc, 512 - sc)],
                                ex[:, ds((diag0 + off) * 512 + sc, 512 - sc)],
                            )
                        state["nsec"] = 4
                return u

            units.append(tail_a())

            def tail_b(h=h, ex=ex, ps_c=ps_c, state=state, qb=qb):
                def u():
                    # denominator: accumulating ones-matmuls over the folded
                    # column-sum sections
                    lt = mm_ps.tile([1, QB], F32, tag="mm", name="ps_l")
                    n_sec = state["nsec"]
                    for j in range(n_sec):
                        nc.tensor.matmul(
                            lt[:],
                            lhsT=ones_ap(),
                            rhs=ex[:, ds(j * 512, 512)],
                            start=(j == 0),
                            stop=(j == n_sec - 1),
                            skip_group_check=True,
                        )
                    rec = lrec_pool.tile([1, QB], F32, tag="r", name="rec")
                    nc.vector.reciprocal(rec[:], lt[:])
                    bc = bc_pool.tile([128, QB], F32, tag="bc", name="bc")
                    nc.gpsimd.partition_broadcast(bc[:], rec[:])
                    with nc.allow_low_precision(reason="ctx bf16, single rounding"):
                        nc.vector.tensor_mul(
                            ctx_sbs[h][:, ts(qb, QB)], ps_c[:], bc[:]
                        )
                return u

            units.append(tail_b())
        return units

    # ---------------- out-proj units for q-block qb ----------------
    # one bf16 staging row-tile [128, D] per seq m-tile; 4 psum drains fill
    # its quarters, then a single DMA writes the row
    def out_units(qb, copy_engine):
        units = []
        for m in range(qb * 4, qb * 4 + 4):
            row = {}

            def mk(m=m, row=row):
                def u_alloc():
                    row["t"] = ostage.tile([128, D], BF16, tag="ot", name="ot")
                return u_alloc

            alloc = mk()
            for n in range(D // QB):
                def u(m=m, n=n, row=row, alloc=alloc):
                    if n == 0:
                        alloc()
                    ps = mm_ps.tile([128, QB], F32, tag="mm", name="ps_o")
                    for k in range(N_DG):
                        nc.tensor.matmul(
                            ps[:],
                            lhsT=ctx_sbs[k][:, ts(m, 128)],
                            rhs=wo_sb[:, ds(k * D + n * QB, QB)],
                            start=(k == 0),
                            stop=(k == N_DG - 1),
                        )
                    ot = row["t"]
                    if copy_engine == "act":
                        nc.scalar.copy(ot[:, ts(n, QB)], ps[:])
                    else:
                        with nc.allow_low_precision(reason="out partial bf16"):
                            nc.vector.tensor_scalar_add(ot[:, ts(n, QB)], ps[:], 0.0)
                    if n == D // QB - 1:
                        nc.sync.dma_start(out[ts(m, 128), :], ot[:])
                units.append(u)
        return units

    # ---------------- schedule ----------------
    xt_sbs = {}
    qt_sbs = {}

    def new_x(s, hlf):
        t = xpool.tile([128, (N_KD // 2) * SL], BF16, tag=f"xt{hlf}", name="xt_sb")
        xt_sbs[(s, hlf)] = t
        return t

    def new_qt(s):
        t = qt_pool.tile([128, N_DG * SL], BF16, tag="qt", name="qt_sb")
        qt_sbs[s] = t
        return t

    # initial DMAs, ordered by first use: wk m0, x0a, wk m1-3, x0b,
    # small consts, wv halves, wq
    load_w_mblock("wk", 0, 1)
    x0a = new_x(0, 0)
    load_xt_half(0, 0, x0a)
    load_w_mblock("wk", 1, 3)
    x0b = new_x(0, 1)
    load_xt_half(0, 1, x0b)
    nc.sync.dma_start(bqk_sb[:], aps["bqk"])
    nc.sync.dma_start(om_sb[:], aps["om"])
    load_w_khalf("wv", 0)
    load_w_khalf("wv", 1)
    load_w_khalf("wq", 0)
    load_w_khalf("wq", 1)

    # window 0: proj slice 0 only; prefetch x1, wo
    u = proj_units(0, x0a, x0b, new_qt(0))
    load_xt_half(1, 0, new_x(1, 0))
    load_xt_half(1, 1, new_x(1, 1))
    load_wo()
    for f in u:
        f()

    # windows 1..3: proj slice s + attn qb s-1 + out qb s-2
    copy_eng = {0: "act", 1: "dve", 2: "dve", 3: "act"}
    for s in range(1, N_SL):
        qb = s - 1
        if s + 1 < N_SL:
            load_xt_half(s + 1, 0, new_x(s + 1, 0))
            load_xt_half(s + 1, 1, new_x(s + 1, 1))
        filler = proj_units(s, xt_sbs[(s, 0)], xt_sbs[(s, 1)], new_qt(s))
        if s >= 2:
            filler += out_units(s - 2, copy_eng[s - 2])
        with tc.tile_pool(name=f"ex{qb}", bufs=2) as ex_pool:
            primary = attn_units(qb, qt_sbs[qb], ex_pool)
            _interleave(primary, filler)

    # tail: attn qb3 + out qb2, then out qb3
    with tc.tile_pool(name="ex3", bufs=2) as ex_pool:
        primary = attn_units(3, qt_sbs[3], ex_pool)
        filler = out_units(2, copy_eng[2])
        _interleave(primary, filler)
    for f in out_units(3, copy_eng[3]):
        f()


def build_program(S=SEQ, D=D_MODEL, DGl=DG, enable_asserts=False):
    nc = bacc.Bacc(
        "TRN2",
        target_bir_lowering=False,
        debug=False,
        enable_asserts=enable_asserts,
        num_devices=N_CORES,
    )
    aps = {
        "xt": nc.dram_tensor("xt", [D, S], BF16, kind="ExternalInput").ap(),
        "wqt": nc.dram_tensor("wqt", [D, DGl], BF16, kind="ExternalInput").ap(),
        "wkt": nc.dram_tensor("wkt", [D, DGl], BF16, kind="ExternalInput").ap(),
        "wvt": nc.dram_tensor("wvt", [D, DGl], BF16, kind="ExternalInput").ap(),
        "wot": nc.dram_tensor("wot", [DGl, D], BF16, kind="ExternalInput").ap(),
        "bqk": nc.dram_tensor("bqk", [128, 8], F32, kind="ExternalInput").ap(),
        "om": nc.dram_tensor("om", [128, 129], BF16, kind="ExternalInput").ap(),
        "out": nc.dram_tensor("out", [S, D], BF16, kind="ExternalOutput").ap(),
    }
    with tile.TileContext(nc) as tc:
        with ExitStack() as ctx:
            _mha_body(ctx, tc, aps, S, D, DGl)
    nc.compile()
    return nc


def make_om():
    """[ones | multiplicative causal band mask (1.0 where p <= j)], bf16."""
    import ml_dtypes

    p = np.arange(128)[:, None]
    j = np.arange(128)[None, :]
    om = np.ones((128, 129), np.float32)
    om[:, 1:] = (p <= j).astype(np.float32)
    return om.astype(ml_dtypes.bfloat16)


def shard_inputs(x, wq, bq, wk, bk, wv, bv, wo, bo):
    """Build the 8 per-core input maps (host-side layout prep, bf16)."""
    import ml_dtypes

    BF = ml_dtypes.bfloat16
    om = make_om()
    xts = [np.ascontiguousarray(np.asarray(x[b], np.float32).T).astype(BF) for b in range(BATCH)]
    in_maps = []
    for c in range(N_CORES):
        b, g = divmod(c, N_GROUPS)
        sl = slice(g * DG, (g + 1) * DG)
        bqk = np.empty((128, 8), np.float32)
        bqk[:, 0:4] = np.asarray(bq, np.float32)[sl].reshape(-1, 128).T
        bqk[:, 4:8] = np.asarray(bk, np.float32)[sl].reshape(-1, 128).T
        in_maps.append(
            {
                "xt": xts[b],
                "wqt": np.ascontiguousarray(np.asarray(wq, np.float32)[sl].T).astype(BF),
                "wkt": np.ascontiguousarray(np.asarray(wk, np.float32)[sl].T).astype(BF),
                "wvt": np.ascontiguousarray(np.asarray(wv, np.float32)[sl].T).astype(BF),
                "wot": np.ascontiguousarray(np.asarray(wo, np.float32)[:, sl].T).astype(BF),
                "bqk": bqk,
                "om": om,
            }
        )
    return in_maps


def out_bias(bv, wo, bo):
    """Host-side constant: bo + bv @ wo^T (softmax rows sum to 1)."""
    return (
        np.asarray(bo, np.float64)
        + np.asarray(bv, np.float64) @ np.asarray(wo, np.float64).T
    ).astype(np.float32)


_NC_CACHE = {}


def get_program():
    if "nc" not in _NC_CACHE:
        _NC_CACHE["nc"] = build_program()
    return _NC_CACHE["nc"]


def run_sharded(inputs, trace=False):
    nc = get_program()
    in_maps = shard_inputs(**inputs)
    res = run_bass_kernel_spmd(nc, in_maps, list(range(N_CORES)), trace=trace)
    bias = out_bias(inputs["bv"], inputs["wo"], inputs["bo"])
    full = np.empty((BATCH, SEQ, D_MODEL), np.float32)
    for b in range(BATCH):
        acc = res.results[b * N_GROUPS]["out"].astype(np.float32)
        for g in range(1, N_GROUPS):
            acc += res.results[b * N_GROUPS + g]["out"].astype(np.float32)
        full[b] = acc + bias
    return full, res


def kernel(**inputs):
    out, _ = run_sharded(inputs, trace=False)
    return out


# revision 43
# speedup vs baseline: 1.1698x; 1.0337x over previous
"""Multi-head causal attention block on 8 Trainium2 NeuronCores.

Sharding: tensor-parallel over heads (4 groups of 4 heads) x data-parallel
over batch (2). Core c -> (batch b=c//4, head-group g=c%4). Each core
computes q/k/v projections for its head group, causal attention for its 4
heads, and a partial output projection; the host sums the 4 partials per
batch and adds (bo + bv @ wo^T) once (softmax rows sum to 1, so the v-bias
contributes exactly bv @ wo^T).

All matmul operands are bf16. The kernel is a single pipelined pass:
window s runs projection slice s, attention q-block s-1, and output
projection for q-block s-2 interleaved at ~850ns granularity so the PE
stream never blocks on Activation-engine exp. The softmax denominator is
built from DVE wide-folds of the exp tiles plus short accumulating
ones-matmuls (cheap on PE), not a full ones-matmul per k-tile.

Self-contained: hardcodes shapes for the 2x2048x2048, 16-head problem.
"""

from contextlib import ExitStack

import numpy as np

import concourse.bass as bass
import concourse.tile as tile
from concourse import bacc, mybir
from concourse.bass import ds, ts
from concourse.bass_utils import run_bass_kernel_spmd

F32 = mybir.dt.float32
BF16 = mybir.dt.bfloat16
ACTF = mybir.ActivationFunctionType

# Full-problem dims
BATCH = 2
SEQ = 2048
D_MODEL = 2048
NUM_HEADS = 16
HEAD_DIM = 128
N_CORES = 8
N_GROUPS = 4  # head-groups (tensor parallel)
DG = D_MODEL // N_GROUPS  # 512 = 4 heads per group
SCALE = 1.0 / float(np.sqrt(HEAD_DIM))

SL = 512  # projection slice width (seq) == attention q-block width
QB = 512
KT = 128
N_SL = SEQ // SL  # 4
N_KD = D_MODEL // 128  # 16 contraction tiles
N_DG = DG // 128  # 4 heads per group


def _interleave(primary, filler):
    """Merge unit lists: spread primary units evenly among filler units.

    Each unit is a zero-arg callable. Emits all units exactly once.
    """
    np_, nf = len(primary), len(filler)
    if np_ == 0:
        for u in filler:
            u()
        return
    if nf == 0:
        for u in primary:
            u()
        return
    fi = 0
    for i, u in enumerate(primary):
        u()
        # after primary unit i, emit filler up to proportional position
        target = (i + 1) * nf // np_
        while fi < target:
            filler[fi]()
            fi += 1
    while fi < nf:
        filler[fi]()
        fi += 1


def _mha_body(ctx, tc, aps, S, D, DGl):
    nc = tc.nc
    n_sl = S // SL
    xt, out = aps["xt"], aps["out"]
    wts = {"wv": aps["wvt"]}

    # ---------------- persistent SBUF tiles ----------------
    consts = ctx.enter_context(tc.tile_pool(name="consts", bufs=1))
    warm = consts.tile([128, 1], F32, name="act_warm")
    nc.vector.memset(warm[:], 0.0)
    nc.scalar.activation(warm[:], warm[:], ACTF.Identity, bias=warm[:, 0:1])
    # merged small constants: one f32 DMA (bq|bk) + one bf16 DMA (ones|mask)
    bqk_sb = consts.tile([128, 2 * N_DG], F32, name="bqk_sb")
    om_sb = consts.tile([128, 129], BF16, name="om_sb")

    def ones_ap():
        return om_sb[:, ds(0, 1)]

    def mask_ap():
        return om_sb[:, ds(1, 128)]

    # weights as per-DMA tiles so readers only wait on the DMA they need:
    # wk/wq as one tile per head m-block (wk m0 further split in k-halves
    # to cut the first-matmul critical path), wv as two k-half tiles
    wpool = ctx.enter_context(tc.tile_pool(name="wpool", bufs=1))
    wk_m0h = [
        wpool.tile([128, (N_KD // 2) * 128], BF16, name=f"wk_m0{i}") for i in range(2)
    ]
    wk_m = [None] + [
        wpool.tile([128, N_KD * 128], BF16, name=f"wk_m{m}") for m in range(1, N_DG)
    ]
    wq_m = [wpool.tile([128, N_KD * 128], BF16, name=f"wq_m{m}") for m in range(N_DG)]
    wv_h = [wpool.tile([128, (N_KD // 2) * DGl], BF16, name=f"wv_h{i}") for i in range(2)]
    wo_sb = wpool.tile([128, N_DG * D], BF16, name="wo_sb")

    def wk_ap(m, k):
        if m == 0:
            t = wk_m0h[k // (N_KD // 2)]
            return t[:, ds((k % (N_KD // 2)) * 128, 128)]
        return wk_m[m][:, ds(k * 128, 128)]

    kt_pool = ctx.enter_context(tc.tile_pool(name="kt_pool", bufs=1))
    kt_res = [kt_pool.tile([128, S], BF16, name=f"kt{h}") for h in range(N_DG)]
    v_res = [kt_pool.tile([128, DGl], BF16, name=f"v{t}") for t in range(S // 128)]
    ctx_sbs = [kt_pool.tile([128, S], BF16, name=f"ctx{h}") for h in range(N_DG)]

    xpool = ctx.enter_context(tc.tile_pool(name="xpool", bufs=2))
    qt_pool = ctx.enter_context(tc.tile_pool(name="qt_pool", bufs=2))
    lrec_pool = ctx.enter_context(tc.tile_pool(name="lrec_pool", bufs=2))
    bc_pool = ctx.enter_context(tc.tile_pool(name="bc_pool", bufs=2))
    ostage = ctx.enter_context(tc.tile_pool(name="ostage", bufs=2))

    mm_ps = ctx.enter_context(tc.tile_pool(name="mm_ps", bufs=2, space="PSUM"))
    sc_ps = ctx.enter_context(tc.tile_pool(name="sc_ps", bufs=2, space="PSUM"))
    c_ps = ctx.enter_context(tc.tile_pool(name="c_ps", bufs=2, space="PSUM"))

    # ---------------- DMA helpers ----------------
    def load_w_mblock(wname, m):
        # host provides [m, p, k*128+j] contiguous layout: 4KB runs per
        # partition (sub-512B runs pay a 2x DMA latency penalty)
        src = aps["wkp" if wname == "wk" else "wqp"]
        if wname == "wk" and m == 0:
            hw_ = (N_KD // 2) * 128
            for i in range(2):
                nc.sync.dma_start(wk_m0h[i][:], src[0, :, ds(i * hw_, hw_)])
            return
        t = (wk_m if wname == "wk" else wq_m)[m]
        nc.sync.dma_start(t[:], src[m])

    def load_wv_khalf(hlf):
        half = N_KD // 2
        nc.sync.dma_start(
            wv_h[hlf][:].rearrange("p (k f) -> p k f", k=half),
            wts["wv"].rearrange("(k p) f -> p k f", p=128)[:, ds(hlf * half, half), :],
        )

    def load_wo():
        nc.sync.dma_start(
            wo_sb[:].rearrange("p (k f) -> p k f", k=N_DG),
            aps["wot"].rearrange("(k p) f -> p k f", p=128),
        )

    def load_xt_q(s, q, t):
        # t is a dedicated quarter-slice tile [128, 4*SL] covering k q*4..q*4+3
        nc.sync.dma_start(
            t[:].rearrange("p (k f) -> p k f", k=4),
            xt[ds(q * 4 * 128, 4 * 128), ts(s, SL)].rearrange(
                "(k p) f -> p k f", p=128
            ),
        )

    # ---------------- projection units for slice s ----------------
    # 12 psum tiles per slice (4 k-m, 4 v-sub, 4 q-m), each 16 matmuls
    # emitted as 4 chunks of 4, plus a drain. x slice is two half tiles
    # (xa: k 0-7, xb: k 8-15) so the first chunks only depend on xa's DMA.
    def proj_units(s, xq, qt_sb):
        units = []

        def xs(k, off=0, w=SL):
            return xq[k // 4][:, ds((k % 4) * SL + off, w)]

        def qk_tile(wname, m, res_ap, bias_off):
            ps = mm_ps.tile([128, SL], F32, tag="mm", name="ps_qk")

            def wap(k):
                if wname == "wk":
                    return wk_ap(m, k)
                return wq_m[m][:, ds(k * 128, 128)]

            def chunk(k0, k1):
                def u():
                    for k in range(k0, k1):
                        nc.tensor.matmul(
                            ps[:],
                            lhsT=wap(k),
                            rhs=xs(k),
                            start=(k == 0),
                            stop=(k == N_KD - 1),
                        )
                return u

            for c in range(4):
                units.append(chunk(c * 4, c * 4 + 4))

            def drain():
                nc.scalar.activation(
                    res_ap, ps[:], ACTF.Identity, bias=bqk_sb[:, ds(bias_off + m, 1)]
                )

            units.append(drain)

        def v_tile(msub):
            ps = mm_ps.tile([128, DGl], F32, tag="mm", name="ps_v")

            def chunk(k0, k1):
                def u():
                    for k in range(k0, k1):
                        nc.tensor.matmul(
                            ps[:],
                            lhsT=xs(k, msub * 128, 128),
                            rhs=wv_h[k // (N_KD // 2)][:, ts(k % (N_KD // 2), DGl)],
                            start=(k == 0),
                            stop=(k == N_KD - 1),
                        )
                return u

            for c in range(4):
                units.append(chunk(c * 4, c * 4 + 4))

            def drain():
                nc.scalar.copy(v_res[s * 4 + msub][:], ps[:])

            units.append(drain)

        for m in range(N_DG):
            qk_tile("wk", m, kt_res[m][:, ts(s, SL)], N_DG)
        for msub in range(4):
            v_tile(msub)
        for m in range(N_DG):
            qk_tile("wq", m, qt_sb[:, ts(m, SL)], 0)
        return units

    # ---------------- attention units for q-block qb ----------------
    def attn_units(qb, qt_sb, ex_pool):
        units = []
        pend_tail = None  # previous head's tail_b, staggered for fold latency
        n_kt = (qb + 1) * 4
        diag0 = qb * 4
        n_pair = n_kt // 2

        for h in range(N_DG):
            ex = ex_pool.tile([128, n_kt * 512], BF16, tag="ex", name="ex")
            ps_c = c_ps.tile([128, QB], F32, tag="c", name="ps_c")
            state = {}

            def sc_of(kt, diag0=diag0):
                off = kt - diag0
                return 0 if off < 0 else off * 128

            def pair_unit(p, h=h, ex=ex, ps_c=ps_c, state=state):
                def u():
                    kts = (2 * p, 2 * p + 1)
                    is_diag = kts[0] >= diag0
                    ps_s = sc_ps.tile([128, 1024], F32, tag="s", name="ps_s")
                    for i, kt in enumerate(kts):
                        sc = sc_of(kt)
                        nc.tensor.matmul(
                            ps_s[:, ds(i * 512 + sc, 512 - sc)],
                            lhsT=kt_res[h][:, ts(kt, 128)],
                            rhs=qt_sb[:, ds(h * SL + sc, 512 - sc)],
                            start=True,
                            stop=True,
                        )
                    # PVs of previous pair (software pipeline)
                    if "prev" in state:
                        pp = state["prev"]
                        for kt in (2 * pp, 2 * pp + 1):
                            sc = sc_of(kt)
                            nc.tensor.matmul(
                                ps_c[:, ds(sc, 512 - sc)],
                                lhsT=v_res[kt][:, ts(h, 128)],
                                rhs=ex[:, ds(kt * 512 + sc, 512 - sc)],
                                start=(kt == 0),
                                stop=(kt == n_kt - 1),
                                skip_group_check=True,
                            )
                    if not is_diag:
                        # merged exp over both (fully written) halves
                        nc.scalar.activation(
                            ex[:, ds(kts[0] * 512, 1024)],
                            ps_s[:],
                            ACTF.Exp,
                            scale=SCALE,
                        )
                    else:
                        # diag tiles: individually trimmed exps (the psum
                        # region left of each sc is unwritten), then zero
                        # the triangular band via mask multiply
                        for i, kt in enumerate(kts):
                            sc = sc_of(kt)
                            nc.scalar.activation(
                                ex[:, ds(kt * 512 + sc, 512 - sc)],
                                ps_s[:, ds(i * 512 + sc, 512 - sc)],
                                ACTF.Exp,
                                scale=SCALE,
                            )
                        for i, kt in enumerate(kts):
                            off = kt - diag0
                            band = kt * 512 + off * 128
                            nc.vector.tensor_mul(
                                ex[:, ds(band, 128)],
                                ex[:, ds(band, 128)],
                                mask_ap(),
                            )
                    state["prev"] = p
                return u

            head_units = [pair_unit(p) for p in range(n_pair)]

            def tail_a(h=h, ex=ex, ps_c=ps_c, state=state):
                def u():
                    # last pair's PVs
                    pp = state["prev"]
                    for kt in (2 * pp, 2 * pp + 1):
                        sc = sc_of(kt)
                        nc.tensor.matmul(
                            ps_c[:, ds(sc, 512 - sc)],
                            lhsT=v_res[kt][:, ts(h, 128)],
                            rhs=ex[:, ds(kt * 512 + sc, 512 - sc)],
                            start=(kt == 0),
                            stop=(kt == n_kt - 1),
                            skip_group_check=True,
                        )
    # fold full sections down to <= 4 on DVE (bf16 2x mode) in
                    # place into ex sections 0..3, then fold the trimmed
                    # diagonal sections on top. All PVs for this head are
                    # done, so in-place ex edits are safe.
                    n_full = diag0
                    with nc.allow_low_precision(reason="colsum fold, <=3 roundings"):
                        if n_full == 0:
                            # qb0: fold trimmed diags onto section 0
                            for off in range(1, 4):
                                sc = off * 128
                                nc.vector.tensor_add(
                                    ex[:, ds(sc, 512 - sc)],
                                    ex[:, ds(sc, 512 - sc)],
                                    ex[:, ds(off * 512 + sc, 512 - sc)],
                                )
                            state["nsec"] = 1
                            return
                        if n_full >= 8:
                            nc.vector.tensor_add(
                                ex[:, ds(0, 2048)],
                                ex[:, ds(0, 2048)],
                                ex[:, ds(2048, 2048)],
                            )
                        if n_full == 12:
                            nc.vector.tensor_add(
                                ex[:, ds(0, 2048)],
                                ex[:, ds(0, 2048)],
                                ex[:, ds(4096, 2048)],
                            )
                        for off in range(4):
                            sc = off * 128
                            nc.vector.tensor_add(
                                ex[:, ds(off * 512 + sc, 512 - sc)],
                                ex[:, ds(off * 512 + sc, 512 - sc)],
                                ex[:, ds((diag0 + off) * 512 + sc, 512 - sc)],
                            )
                        if qb < 3:
                            # fold 4 -> 1 to save three ones-matmuls on PE
                            # (qb3 keeps 4 sections: its window has less
                            # DVE slack)
                            nc.vector.tensor_add(
                                ex[:, ds(0, 1024)],
                                ex[:, ds(0, 1024)],
                                ex[:, ds(1024, 1024)],
                            )
                            nc.vector.tensor_add(
                                ex[:, ds(0, 512)],
                                ex[:, ds(0, 512)],
                                ex[:, ds(512, 512)],
                            )
                            state["nsec"] = 1
                        else:
                            state["nsec"] = 4
                return u

            head_units.append(tail_a())

            def tail_b(h=h, ex=ex, ps_c=ps_c, state=state, qb=qb):
                def u():
                    # denominator: accumulating ones-matmuls over the folded
                    # column-sum sections
                    lt = mm_ps.tile([1, QB], F32, tag="mm", name="ps_l")
                    n_sec = state["nsec"]
                    for j in range(n_sec):
                        nc.tensor.matmul(
                            lt[:],
                            lhsT=ones_ap(),
                            rhs=ex[:, ds(j * 512, 512)],
                            start=(j == 0),
                            stop=(j == n_sec - 1),
                            skip_group_check=True,
                        )
                    rec = lrec_pool.tile([1, QB], F32, tag="r", name="rec")
                    nc.vector.reciprocal(rec[:], lt[:])
                    bc = bc_pool.tile([128, QB], F32, tag="bc", name="bc")
                    nc.gpsimd.partition_broadcast(bc[:], rec[:])
                    with nc.allow_low_precision(reason="ctx bf16, single rounding"):
                        nc.vector.tensor_mul(
                            ctx_sbs[h][:, ts(qb, QB)], ps_c[:], bc[:]
                        )
                return u

            if pend_tail is not None:
                head_units.insert(1, pend_tail)
            pend_tail = tail_b()
            units += head_units
        units.append(pend_tail)
        return units

    # ---------------- out-proj units for q-block qb ----------------
    # one bf16 staging row-tile [128, D] per seq m-tile; 4 psum drains fill
    # its quarters, then a single DMA writes the row
    def out_units(qb, copy_engine):
        units = []
        for m in range(qb * 4, qb * 4 + 4):
            row = {}

            def mk(m=m, row=row):
                def u_alloc():
                    row["t"] = ostage.tile([128, D], BF16, tag="ot", name="ot")
                return u_alloc

            alloc = mk()
            for n in range(D // QB):
                def u(m=m, n=n, row=row, alloc=alloc):
                    if n == 0:
                        alloc()
                    ps = mm_ps.tile([128, QB], F32, tag="mm", name="ps_o")
                    for k in range(N_DG):
                        nc.tensor.matmul(
                            ps[:],
                            lhsT=ctx_sbs[k][:, ts(m, 128)],
                            rhs=wo_sb[:, ds(k * D + n * QB, QB)],
                            start=(k == 0),
                            stop=(k == N_DG - 1),
                        )
                    ot = row["t"]
                    if copy_engine == "act":
                        nc.scalar.copy(ot[:, ts(n, QB)], ps[:])
                    else:
                        with nc.allow_low_precision(reason="out partial bf16"):
                            nc.vector.tensor_scalar_add(ot[:, ts(n, QB)], ps[:], 0.0)
                    if n % 2 == 1:
                        nc.sync.dma_start(
                            out[ts(m, 128), ds((n - 1) * QB, 2 * QB)],
                            ot[:, ds((n - 1) * QB, 2 * QB)],
                        )
                units.append(u)
        return units

    # ---------------- schedule ----------------
    xt_sbs = {}
    qt_sbs = {}

    def new_xq(s):
        ts_ = [
            xpool.tile([128, 4 * SL], BF16, tag=f"xt{q}", name="xt_sb")
            for q in range(4)
        ]
        xt_sbs[s] = ts_
        return ts_

    def load_x(s, xq):
        for q in range(4):
            load_xt_q(s, q, xq[q])

    def new_qt(s):
        t = qt_pool.tile([128, N_DG * SL], BF16, tag="qt", name="qt_sb")
        qt_sbs[s] = t
        return t

    # initial DMAs, interleaved x-quarters and wk blocks so the first
    # matmuls' dependencies land earliest on the serialized DMA resource
    x0 = new_xq(0)
    load_xt_q(0, 0, x0[0])
    load_w_mblock("wk", 0)  # two half DMAs
    load_xt_q(0, 1, x0[1])
    nc.sync.dma_start(bqk_sb[:], aps["bqk"])
    nc.sync.dma_start(om_sb[:], aps["om"])
    load_xt_q(0, 2, x0[2])
    load_xt_q(0, 3, x0[3])
    load_w_mblock("wk", 1)
    load_w_mblock("wk", 2)
    load_w_mblock("wk", 3)
    load_wv_khalf(0)
    load_wv_khalf(1)
    for m in range(N_DG):
        load_w_mblock("wq", m)

    # window 0: proj slice 0 only; prefetch x1, wo
    u = proj_units(0, x0, new_qt(0))
    load_x(1, new_xq(1))
    load_wo()
    for f in u:
        f()

    # windows 1..3: proj slice s + attn qb s-1 + out qb s-2
    copy_eng = {0: "act", 1: "dve", 2: "dve", 3: "act"}
    for s in range(1, N_SL):
        qb = s - 1
        if s + 1 < N_SL:
            load_x(s + 1, new_xq(s + 1))
        filler = proj_units(s, xt_sbs[s], new_qt(s))
        if s >= 2:
            filler += out_units(s - 2, copy_eng[s - 2])
        with tc.tile_pool(name=f"ex{qb}", bufs=2) as ex_pool:
            primary = attn_units(qb, qt_sbs[qb], ex_pool)
            _interleave(primary, filler)

    # tail: attn qb3 + out qb2 (holding back a few units to cover the
    # last head's denominator-chain latency), then out qb3
    with tc.tile_pool(name="ex3", bufs=2) as ex_pool:
        primary = attn_units(3, qt_sbs[3], ex_pool)
        filler = out_units(2, copy_eng[2])
        _interleave(primary, filler[:-4])
        for f in filler[-4:]:
            f()
    for f in out_units(3, copy_eng[3]):
        f()


def build_program(S=SEQ, D=D_MODEL, DGl=DG, enable_asserts=False):
    nc = bacc.Bacc(
        "TRN2",
        target_bir_lowering=False,
        debug=False,
        enable_asserts=enable_asserts,
        num_devices=N_CORES,
    )
    aps = {
        "xt": nc.dram_tensor("xt", [D, S], BF16, kind="ExternalInput").ap(),
        "wqp": nc.dram_tensor(
            "wqp", [N_DG, 128, (D // 128) * 128], BF16, kind="ExternalInput"
        ).ap(),
        "wkp": nc.dram_tensor(
            "wkp", [N_DG, 128, (D // 128) * 128], BF16, kind="ExternalInput"
        ).ap(),
        "wvt": nc.dram_tensor("wvt", [D, DGl], BF16, kind="ExternalInput").ap(),
        "wot": nc.dram_tensor("wot", [DGl, D], BF16, kind="ExternalInput").ap(),
        "bqk": nc.dram_tensor("bqk", [128, 8], F32, kind="ExternalInput").ap(),
        "om": nc.dram_tensor("om", [128, 129], BF16, kind="ExternalInput").ap(),
        "out": nc.dram_tensor("out", [S, D], BF16, kind="ExternalOutput").ap(),
    }
    with tile.TileContext(nc) as tc:
        with ExitStack() as ctx:
            _mha_body(ctx, tc, aps, S, D, DGl)
    nc.compile()
    return nc


def make_om():
    """[ones | multiplicative causal band mask (1.0 where p <= j)], bf16."""
    import ml_dtypes

    p = np.arange(128)[:, None]
    j = np.arange(128)[None, :]
    om = np.ones((128, 129), np.float32)
    om[:, 1:] = (p <= j).astype(np.float32)
    return om.astype(ml_dtypes.bfloat16)


def wm_layout(w, sl):
    """Per-m-block DMA-friendly layout: wmp[m, p, k*128+j] = w[sl][m*128+j, k*128+p]."""
    import ml_dtypes

    w_sl = np.asarray(w, np.float32)[sl]  # [DG, D]
    arr = w_sl.reshape(N_DG, 128, D_MODEL // 128, 128)  # [m, j, k, p]
    return np.ascontiguousarray(arr.transpose(0, 3, 2, 1)).reshape(
        N_DG, 128, -1
    ).astype(ml_dtypes.bfloat16)


def shard_inputs(x, wq, bq, wk, bk, wv, bv, wo, bo):
    """Build the 8 per-core input maps (host-side layout prep, bf16)."""
    import ml_dtypes

    BF = ml_dtypes.bfloat16
    om = make_om()
    xts = [np.ascontiguousarray(np.asarray(x[b], np.float32).T).astype(BF) for b in range(BATCH)]
    in_maps = []
    for c in range(N_CORES):
        b, g = divmod(c, N_GROUPS)
        sl = slice(g * DG, (g + 1) * DG)
        bqk = np.empty((128, 8), np.float32)
        bqk[:, 0:4] = np.asarray(bq, np.float32)[sl].reshape(-1, 128).T
        bqk[:, 4:8] = np.asarray(bk, np.float32)[sl].reshape(-1, 128).T
        in_maps.append(
            {
                "xt": xts[b],
                "wqp": wm_layout(wq, sl),
                "wkp": wm_layout(wk, sl),
                "wvt": np.ascontiguousarray(np.asarray(wv, np.float32)[sl].T).astype(BF),
                "wot": np.ascontiguousarray(np.asarray(wo, np.float32)[:, sl].T).astype(BF),
                "bqk": bqk,
                "om": om,
            }
        )
    return in_maps


def out_bias(bv, wo, bo):
    """Host-side constant: bo + bv @ wo^T (softmax rows sum to 1)."""
    return (
        np.asarray(bo, np.float64)
        + np.asarray(bv, np.float64) @ np.asarray(wo, np.float64).T
    ).astype(np.float32)


_NC_CACHE = {}


def get_program():
    if "nc" not in _NC_CACHE:
        _NC_CACHE["nc"] = build_program()
    return _NC_CACHE["nc"]


def run_sharded(inputs, trace=False):
    nc = get_program()
    in_maps = shard_inputs(**inputs)
    res = run_bass_kernel_spmd(nc, in_maps, list(range(N_CORES)), trace=trace)
    bias = out_bias(inputs["bv"], inputs["wo"], inputs["bo"])
    full = np.empty((BATCH, SEQ, D_MODEL), np.float32)
    for b in range(BATCH):
        acc = res.results[b * N_GROUPS]["out"].astype(np.float32)
        for g in range(1, N_GROUPS):
            acc += res.results[b * N_GROUPS + g]["out"].astype(np.float32)
        full[b] = acc + bias
    return full, res


def kernel(**inputs):
    out, _ = run_sharded(inputs, trace=False)
    return out


# revision 63
# speedup vs baseline: 1.1714x; 1.0014x over previous
"""Multi-head causal attention block on 8 Trainium2 NeuronCores.

Sharding: tensor-parallel over heads (4 groups of 4 heads) x data-parallel
over batch (2). Core c -> (batch b=c//4, head-group g=c%4). Each core
computes q/k/v projections for its head group, causal attention for its 4
heads, and a partial output projection; the host sums the 4 partials per
batch and adds (bo + bv @ wo^T) once (softmax rows sum to 1, so the v-bias
contributes exactly bv @ wo^T).

All matmul operands are bf16. The kernel is a single pipelined pass:
window s runs projection slice s, attention q-block s-1, and output
projection for q-block s-2 interleaved at ~850ns granularity so the PE
stream never blocks on Activation-engine exp. The softmax denominator is
built from DVE wide-folds of the exp tiles plus short accumulating
ones-matmuls (cheap on PE), not a full ones-matmul per k-tile.

Self-contained: hardcodes shapes for the 2x2048x2048, 16-head problem.
"""

from contextlib import ExitStack

import numpy as np

import concourse.bass as bass
import concourse.tile as tile
from concourse import bacc, mybir
from concourse.bass import ds, ts
from concourse.bass_utils import run_bass_kernel_spmd

F32 = mybir.dt.float32
BF16 = mybir.dt.bfloat16
ACTF = mybir.ActivationFunctionType

# Full-problem dims
BATCH = 2
SEQ = 2048
D_MODEL = 2048
NUM_HEADS = 16
HEAD_DIM = 128
N_CORES = 8
N_GROUPS = 4  # head-groups (tensor parallel)
DG = D_MODEL // N_GROUPS  # 512 = 4 heads per group
SCALE = 1.0 / float(np.sqrt(HEAD_DIM))

SL = 512  # projection slice width (seq) == attention q-block width
QB = 512
KT = 128
N_SL = SEQ // SL  # 4
N_KD = D_MODEL // 128  # 16 contraction tiles
N_DG = DG // 128  # 4 heads per group


def _interleave(primary, filler):
    """Merge unit lists: spread primary units evenly among filler units.

    Each unit is a zero-arg callable. Emits all units exactly once.
    """
    np_, nf = len(primary), len(filler)
    if np_ == 0:
        for u in filler:
            u()
        return
    if nf == 0:
        for u in primary:
            u()
        return
    fi = 0
    for i, u in enumerate(primary):
        u()
        # after primary unit i, emit filler up to proportional position
        target = (i + 1) * nf // np_
        while fi < target:
            filler[fi]()
            fi += 1
    while fi < nf:
        filler[fi]()
        fi += 1


def _mha_body(ctx, tc, aps, S, D, DGl):
    nc = tc.nc
    n_sl = S // SL
    xt, out = aps["xt"], aps["out"]
    wts = {"wv": aps["wvt"]}

    # ---------------- persistent SBUF tiles ----------------
    consts = ctx.enter_context(tc.tile_pool(name="consts", bufs=1))
    warm = consts.tile([128, 1], F32, name="act_warm")
    nc.vector.memset(warm[:], 0.0)
    nc.scalar.activation(warm[:], warm[:], ACTF.Identity, bias=warm[:, 0:1])
    # merged small constants: one f32 DMA (bq|bk) + one bf16 DMA (ones|mask)
    bqk_sb = consts.tile([128, 2 * N_DG], F32, name="bqk_sb")
    om_sb = consts.tile([128, 129], BF16, name="om_sb")

    def ones_ap():
        return om_sb[:, ds(0, 1)]

    def mask_ap():
        return om_sb[:, ds(1, 128)]

    # weights as per-DMA tiles so readers only wait on the DMA they need:
    # wk/wq as one tile per head m-block (wk m0 further split in k-halves
    # to cut the first-matmul critical path), wv as two k-half tiles
    wpool = ctx.enter_context(tc.tile_pool(name="wpool", bufs=1))
    wk_m0p = [
        wpool.tile([128, nk * 128], BF16, name=f"wk_m0{i}")
        for i, nk in enumerate((8, 8))
    ]
    _wk_m0_k0 = (0, 8)
    wk_m = [None] + [
        wpool.tile([128, N_KD * 128], BF16, name=f"wk_m{m}") for m in range(1, N_DG)
    ]
    wq_m = [wpool.tile([128, N_KD * 128], BF16, name=f"wq_m{m}") for m in range(N_DG)]
    wv_h = [wpool.tile([128, (N_KD // 2) * DGl], BF16, name=f"wv_h{i}") for i in range(2)]
    wo_sb = wpool.tile([128, N_DG * D], BF16, name="wo_sb")

    def wk_ap(m, k):
        if m == 0:
            i = 0 if k < 8 else 1
            return wk_m0p[i][:, ds((k - _wk_m0_k0[i]) * 128, 128)]
        return wk_m[m][:, ds(k * 128, 128)]

    kt_pool = ctx.enter_context(tc.tile_pool(name="kt_pool", bufs=1))
    kt_res = [kt_pool.tile([128, S], BF16, name=f"kt{h}") for h in range(N_DG)]
    v_res = [kt_pool.tile([128, DGl], BF16, name=f"v{t}") for t in range(S // 128)]
    ctx_sbs = [kt_pool.tile([128, S], BF16, name=f"ctx{h}") for h in range(N_DG)]

    xpool = ctx.enter_context(tc.tile_pool(name="xpool", bufs=2))
    qt_pool = ctx.enter_context(tc.tile_pool(name="qt_pool", bufs=2))
    lrec_pool = ctx.enter_context(tc.tile_pool(name="lrec_pool", bufs=2))
    bc_pool = ctx.enter_context(tc.tile_pool(name="bc_pool", bufs=2))
    ostage = ctx.enter_context(tc.tile_pool(name="ostage", bufs=2))

    mm_ps = ctx.enter_context(tc.tile_pool(name="mm_ps", bufs=2, space="PSUM"))
    sc_ps = ctx.enter_context(tc.tile_pool(name="sc_ps", bufs=2, space="PSUM"))
    c_ps = ctx.enter_context(tc.tile_pool(name="c_ps", bufs=2, space="PSUM"))

    # ---------------- DMA helpers ----------------
    def load_w_mblock(wname, m):
        # host provides [m, p, k*128+j] contiguous layout: 4KB runs per
        # partition (sub-512B runs pay a 2x DMA latency penalty)
        src = aps["wkp" if wname == "wk" else "wqp"]
        if wname == "wk" and m == 0:
            for i, (k0, nk) in enumerate(zip(_wk_m0_k0, (8, 8))):
                nc.sync.dma_start(wk_m0p[i][:], src[0, :, ds(k0 * 128, nk * 128)])
            return
        t = (wk_m if wname == "wk" else wq_m)[m]
        nc.sync.dma_start(t[:], src[m])

    def load_wv_khalf(hlf):
        half = N_KD // 2
        nc.sync.dma_start(
            wv_h[hlf][:].rearrange("p (k f) -> p k f", k=half),
            wts["wv"].rearrange("(k p) f -> p k f", p=128)[:, ds(hlf * half, half), :],
        )

    def load_wo():
        nc.sync.dma_start(
            wo_sb[:].rearrange("p (k f) -> p k f", k=N_DG),
            aps["wot"].rearrange("(k p) f -> p k f", p=128),
        )

    def load_xt_range(s, k0, nk, t):
        # t is a dedicated tile [128, nk*SL] covering k-tiles k0..k0+nk
        nc.sync.dma_start(
            t[:].rearrange("p (k f) -> p k f", k=nk),
            xt[ds(k0 * 128, nk * 128), ts(s, SL)].rearrange("(k p) f -> p k f", p=128),
        )

    # ---------------- projection units for slice s ----------------
    # 12 psum tiles per slice (4 k-m, 4 v-sub, 4 q-m), each 16 matmuls
    # emitted as 4 chunks of 4, plus a drain. x slice is two half tiles
    # (xa: k 0-7, xb: k 8-15) so the first chunks only depend on xa's DMA.
    def proj_units(s, xtiles, qt_sb):
        units = []

        def xs(k, off=0, w=SL):
            for t, k0, nk in xtiles:
                if k0 <= k < k0 + nk:
                    return t[:, ds((k - k0) * SL + off, w)]
            raise AssertionError(k)

        def qk_tile(wname, m, res_ap, bias_off):
            ps = mm_ps.tile([128, SL], F32, tag="mm", name="ps_qk")

            def wap(k):
                if wname == "wk":
                    return wk_ap(m, k)
                return wq_m[m][:, ds(k * 128, 128)]

            def chunk(k0, k1):
                def u():
                    for k in range(k0, k1):
                        nc.tensor.matmul(
                            ps[:],
                            lhsT=wap(k),
                            rhs=xs(k),
                            start=(k == 0),
                            stop=(k == N_KD - 1),
                        )
                return u

            for c in range(4):
                units.append(chunk(c * 4, c * 4 + 4))

            def drain():
                nc.scalar.activation(
                    res_ap, ps[:], ACTF.Identity, bias=bqk_sb[:, ds(bias_off + m, 1)]
                )

            units.append(drain)

        def v_tile(msub):
            ps = mm_ps.tile([128, DGl], F32, tag="mm", name="ps_v")

            def chunk(k0, k1):
                def u():
                    for k in range(k0, k1):
                        nc.tensor.matmul(
                            ps[:],
                            lhsT=xs(k, msub * 128, 128),
                            rhs=wv_h[k // (N_KD // 2)][:, ts(k % (N_KD // 2), DGl)],
                            start=(k == 0),
                            stop=(k == N_KD - 1),
                        )
                return u

            for c in range(4):
                units.append(chunk(c * 4, c * 4 + 4))

            def drain():
                nc.scalar.copy(v_res[s * 4 + msub][:], ps[:])

            units.append(drain)

        for m in range(N_DG):
            qk_tile("wk", m, kt_res[m][:, ts(s, SL)], N_DG)
        for msub in range(4):
            v_tile(msub)
        for m in range(N_DG):
            qk_tile("wq", m, qt_sb[:, ts(m, SL)], 0)
        return units

    # ---------------- attention units for q-block qb ----------------
    def attn_units(qb, qt_sb, ex_pool):
        units = []
        pend_tail = None  # previous head's tail_b, staggered for fold latency
        n_kt = (qb + 1) * 4
        diag0 = qb * 4
        n_pair = n_kt // 2

        for h in range(N_DG):
            ex = ex_pool.tile([128, n_kt * 512], BF16, tag="ex", name="ex")
            ps_c = c_ps.tile([128, QB], F32, tag="c", name="ps_c")
            state = {}

            def sc_of(kt, diag0=diag0):
                off = kt - diag0
                return 0 if off < 0 else off * 128

            def pair_unit(p, h=h, ex=ex, ps_c=ps_c, state=state):
                def u():
                    kts = (2 * p, 2 * p + 1)
                    is_diag = kts[0] >= diag0
                    ps_s = sc_ps.tile([128, 1024], F32, tag="s", name="ps_s")
                    for i, kt in enumerate(kts):
                        sc = sc_of(kt)
                        nc.tensor.matmul(
                            ps_s[:, ds(i * 512 + sc, 512 - sc)],
                            lhsT=kt_res[h][:, ts(kt, 128)],
                            rhs=qt_sb[:, ds(h * SL + sc, 512 - sc)],
                            start=True,
                            stop=True,
                        )
                    # PVs of previous pair (software pipeline)
                    if "prev" in state:
                        pp = state["prev"]
                        for kt in (2 * pp, 2 * pp + 1):
                            sc = sc_of(kt)
                            nc.tensor.matmul(
                                ps_c[:, ds(sc, 512 - sc)],
                                lhsT=v_res[kt][:, ts(h, 128)],
                                rhs=ex[:, ds(kt * 512 + sc, 512 - sc)],
                                start=(kt == 0),
                                stop=(kt == n_kt - 1),
                                skip_group_check=True,
                            )
                    if not is_diag:
                        # merged exp over both (fully written) halves
                        nc.scalar.activation(
                            ex[:, ds(kts[0] * 512, 1024)],
                            ps_s[:],
                            ACTF.Exp,
                            scale=SCALE,
                        )
                    else:
                        # diag tiles: individually trimmed exps (the psum
                        # region left of each sc is unwritten), then zero
                        # the triangular band via mask multiply
                        for i, kt in enumerate(kts):
                            sc = sc_of(kt)
                            nc.scalar.activation(
                                ex[:, ds(kt * 512 + sc, 512 - sc)],
                                ps_s[:, ds(i * 512 + sc, 512 - sc)],
                                ACTF.Exp,
                                scale=SCALE,
                            )
                        for i, kt in enumerate(kts):
                            off = kt - diag0
                            band = kt * 512 + off * 128
                            nc.vector.tensor_mul(
                                ex[:, ds(band, 128)],
                                ex[:, ds(band, 128)],
                                mask_ap(),
                            )
                    state["prev"] = p
                return u

            head_units = [pair_unit(p) for p in range(n_pair)]

            def tail_a(h=h, ex=ex, ps_c=ps_c, state=state):
                def u():
                    # last pair's PVs
                    pp = state["prev"]
                    for kt in (2 * pp, 2 * pp + 1):
                        sc = sc_of(kt)
                        nc.tensor.matmul(
                            ps_c[:, ds(sc, 512 - sc)],
                            lhsT=v_res[kt][:, ts(h, 128)],
                            rhs=ex[:, ds(kt * 512 + sc, 512 - sc)],
                            start=(kt == 0),
                            stop=(kt == n_kt - 1),
                            skip_group_check=True,
                        )
    # fold full sections down to <= 4 on DVE (bf16 2x mode) in
                    # place into ex sections 0..3, then fold the trimmed
                    # diagonal sections on top. All PVs for this head are
                    # done, so in-place ex edits are safe.
                    n_full = diag0
                    with nc.allow_low_precision(reason="colsum fold, <=3 roundings"):
                        if n_full == 0:
                            # qb0: fold trimmed diags onto section 0
                            for off in range(1, 4):
                                sc = off * 128
                                nc.vector.tensor_add(
                                    ex[:, ds(sc, 512 - sc)],
                                    ex[:, ds(sc, 512 - sc)],
                                    ex[:, ds(off * 512 + sc, 512 - sc)],
                                )
                            state["nsec"] = 1
                            return
                        if n_full >= 8:
                            nc.vector.tensor_add(
                                ex[:, ds(0, 2048)],
                                ex[:, ds(0, 2048)],
                                ex[:, ds(2048, 2048)],
                            )
                        if n_full == 12:
                            nc.vector.tensor_add(
                                ex[:, ds(0, 2048)],
                                ex[:, ds(0, 2048)],
                                ex[:, ds(4096, 2048)],
                            )
                        for off in range(4):
                            sc = off * 128
                            nc.vector.tensor_add(
                                ex[:, ds(off * 512 + sc, 512 - sc)],
                                ex[:, ds(off * 512 + sc, 512 - sc)],
                                ex[:, ds((diag0 + off) * 512 + sc, 512 - sc)],
                            )
                        if qb < 3:
                            # fold 4 -> 1 to save three ones-matmuls on PE
                            # (qb3 keeps 4 sections: the longer DVE chain
                            # would delay the tail's norm path)
                            nc.vector.tensor_add(
                                ex[:, ds(0, 1024)],
                                ex[:, ds(0, 1024)],
                                ex[:, ds(1024, 1024)],
                            )
                            nc.vector.tensor_add(
                                ex[:, ds(0, 512)],
                                ex[:, ds(0, 512)],
                                ex[:, ds(512, 512)],
                            )
                            state["nsec"] = 1
                        else:
                            state["nsec"] = 4
                return u

            head_units.append(tail_a())

            def tail_b(h=h, ex=ex, ps_c=ps_c, state=state, qb=qb):
                def u():
                    # denominator: accumulating ones-matmuls over the folded
                    # column-sum sections
                    lt = mm_ps.tile([1, QB], F32, tag="mm", name="ps_l")
                    n_sec = state["nsec"]
                    for j in range(n_sec):
                        nc.tensor.matmul(
                            lt[:],
                            lhsT=ones_ap(),
                            rhs=ex[:, ds(j * 512, 512)],
                            start=(j == 0),
                            stop=(j == n_sec - 1),
                            skip_group_check=True,
                        )
                    rec = lrec_pool.tile([1, QB], F32, tag="r", name="rec")
                    nc.vector.reciprocal(rec[:], lt[:])
                    bc = bc_pool.tile([128, QB], F32, tag="bc", name="bc")
                    nc.gpsimd.partition_broadcast(bc[:], rec[:])
                    with nc.allow_low_precision(reason="ctx bf16, single rounding"):
                        nc.vector.tensor_mul(
                            ctx_sbs[h][:, ts(qb, QB)], ps_c[:], bc[:]
                        )
                return u

            if pend_tail is not None:
                head_units.insert(1, pend_tail)
            pend_tail = tail_b()
            units += head_units
        units.append(pend_tail)
        return units

    # ---------------- out-proj units for q-block qb ----------------
    # one bf16 staging row-tile [128, D] per seq m-tile; 4 psum drains fill
    # its quarters, then a single DMA writes the row
    def out_units(qb, copy_engine):
        units = []
        for m in range(qb * 4, qb * 4 + 4):
            row = {}

            def mk(m=m, row=row):
                def u_alloc():
                    row["t"] = ostage.tile([128, D], BF16, tag="ot", name="ot")
                return u_alloc

            alloc = mk()
            for n in range(D // QB):
                def u(m=m, n=n, row=row, alloc=alloc):
                    if n == 0:
                        alloc()
                    ps = mm_ps.tile([128, QB], F32, tag="mm", name="ps_o")
                    for k in range(N_DG):
                        nc.tensor.matmul(
                            ps[:],
                            lhsT=ctx_sbs[k][:, ts(m, 128)],
                            rhs=wo_sb[:, ds(k * D + n * QB, QB)],
                            start=(k == 0),
                            stop=(k == N_DG - 1),
                        )
                    ot = row["t"]
                    eng = copy_engine
                    if eng == "alt":
                        eng = "act" if n % 2 == 0 else "dve"
                    if eng == "act":
                        nc.scalar.copy(ot[:, ts(n, QB)], ps[:])
                    else:
                        with nc.allow_low_precision(reason="out partial bf16"):
                            nc.vector.tensor_scalar_add(ot[:, ts(n, QB)], ps[:], 0.0)
                    if m == SEQ // 128 - 1 and n >= 2:
                        # final row: quarter DMAs to shorten the end drain
                        nc.sync.dma_start(
                            out[ts(m, 128), ts(n, QB)], ot[:, ts(n, QB)]
                        )
                    elif n % 2 == 1:
                        nc.sync.dma_start(
                            out[ts(m, 128), ds((n - 1) * QB, 2 * QB)],
                            ot[:, ds((n - 1) * QB, 2 * QB)],
                        )
                units.append(u)
        return units

    # ---------------- schedule ----------------
    xt_sbs = {}
    qt_sbs = {}

    def new_xq(s):
        ts_ = [
            (xpool.tile([128, 4 * SL], BF16, tag=f"xt{q}", name="xt_sb"), q * 4, 4)
            for q in range(4)
        ]
        xt_sbs[s] = ts_
        return ts_

    def load_x(s, xtiles):
        for t, k0, nk in xtiles:
            load_xt_range(s, k0, nk, t)

    def new_qt(s):
        t = qt_pool.tile([128, N_DG * SL], BF16, tag="qt", name="qt_sb")
        qt_sbs[s] = t
        return t

    # initial DMAs, interleaved x pieces and wk blocks so the first
    # matmuls' dependencies land earliest on the serialized DMA resource.
    # Slice 0's first k-quarter is two eighth tiles (scoped pool) and wk m0
    # is quartered: the first matmul only waits ~2 small transfers.
    x0 = new_xq(0)
    load_xt_range(0, 0, 4, x0[0][0])
    load_w_mblock("wk", 0)
    load_xt_range(0, 4, 4, x0[1][0])
    nc.sync.dma_start(bqk_sb[:], aps["bqk"])
    nc.sync.dma_start(om_sb[:], aps["om"])
    load_xt_range(0, 8, 4, x0[2][0])
    load_xt_range(0, 12, 4, x0[3][0])
    load_w_mblock("wk", 1)
    load_w_mblock("wk", 2)
    load_w_mblock("wk", 3)
    load_wv_khalf(0)
    load_wv_khalf(1)
    for m in range(N_DG):
        load_w_mblock("wq", m)

    # window 0: proj slice 0 only; prefetch x1, wo
    u = proj_units(0, x0, new_qt(0))
    load_x(1, new_xq(1))
    load_wo()
    for f in u:
        f()

    # windows 1..3: proj slice s + attn qb s-1 + out qb s-2
    copy_eng = {0: "act", 1: "dve", 2: "dve", 3: "alt"}
    for s in range(1, N_SL):
        qb = s - 1
        if s + 1 < N_SL:
            load_x(s + 1, new_xq(s + 1))
        filler = proj_units(s, xt_sbs[s], new_qt(s))
        if s >= 2:
            filler += out_units(s - 2, copy_eng[s - 2])
        with tc.tile_pool(name=f"ex{qb}", bufs=2) as ex_pool:
            primary = attn_units(qb, qt_sbs[qb], ex_pool)
            _interleave(primary, filler)

    # tail: attn qb3 + out qb2 (holding back a few units to cover the
    # last head's denominator-chain latency), then out qb3
    with tc.tile_pool(name="ex3", bufs=2) as ex_pool:
        primary = attn_units(3, qt_sbs[3], ex_pool)
        filler = out_units(2, copy_eng[2])
        _interleave(primary, filler[:-4])
        for f in filler[-4:]:
            f()
    for f in out_units(3, copy_eng[3]):
        f()


def build_program(S=SEQ, D=D_MODEL, DGl=DG, enable_asserts=False):
    nc = bacc.Bacc(
        "TRN2",
        target_bir_lowering=False,
        debug=False,
        enable_asserts=enable_asserts,
        num_devices=N_CORES,
    )
    aps = {
        "xt": nc.dram_tensor("xt", [D, S], BF16, kind="ExternalInput").ap(),
        "wqp": nc.dram_tensor(
            "wqp", [N_DG, 128, (D // 128) * 128], BF16, kind="ExternalInput"
        ).ap(),
        "wkp": nc.dram_tensor(
            "wkp", [N_DG, 128, (D // 128) * 128], BF16, kind="ExternalInput"
        ).ap(),
        "wvt": nc.dram_tensor("wvt", [D, DGl], BF16, kind="ExternalInput").ap(),
        "wot": nc.dram_tensor("wot", [DGl, D], BF16, kind="ExternalInput").ap(),
        "bqk": nc.dram_tensor("bqk", [128, 8], F32, kind="ExternalInput").ap(),
        "om": nc.dram_tensor("om", [128, 129], BF16, kind="ExternalInput").ap(),
        "out": nc.dram_tensor("out", [S, D], BF16, kind="ExternalOutput").ap(),
    }
    with tile.TileContext(nc) as tc:
        with ExitStack() as ctx:
            _mha_body(ctx, tc, aps, S, D, DGl)
    nc.compile()
    return nc


def make_om():
    """[ones | multiplicative causal band mask (1.0 where p <= j)], bf16."""
    import ml_dtypes

    p = np.arange(128)[:, None]
    j = np.arange(128)[None, :]
    om = np.ones((128, 129), np.float32)
    om[:, 1:] = (p <= j).astype(np.float32)
    return om.astype(ml_dtypes.bfloat16)


def wm_layout(w, sl):
    """Per-m-block DMA-friendly layout: wmp[m, p, k*128+j] = w[sl][m*128+j, k*128+p]."""
    import ml_dtypes

    w_sl = np.asarray(w, np.float32)[sl]  # [DG, D]
    arr = w_sl.reshape(N_DG, 128, D_MODEL // 128, 128)  # [m, j, k, p]
    return np.ascontiguousarray(arr.transpose(0, 3, 2, 1)).reshape(
        N_DG, 128, -1
    ).astype(ml_dtypes.bfloat16)


def shard_inputs(x, wq, bq, wk, bk, wv, bv, wo, bo):
    """Build the 8 per-core input maps (host-side layout prep, bf16)."""
    import ml_dtypes

    BF = ml_dtypes.bfloat16
    om = make_om()
    xts = [np.ascontiguousarray(np.asarray(x[b], np.float32).T).astype(BF) for b in range(BATCH)]
    in_maps = []
    for c in range(N_CORES):
        b, g = divmod(c, N_GROUPS)
        sl = slice(g * DG, (g + 1) * DG)
        bqk = np.empty((128, 8), np.float32)
        bqk[:, 0:4] = np.asarray(bq, np.float32)[sl].reshape(-1, 128).T
        bqk[:, 4:8] = np.asarray(bk, np.float32)[sl].reshape(-1, 128).T
        in_maps.append(
            {
                "xt": xts[b],
                "wqp": wm_layout(wq, sl),
                "wkp": wm_layout(wk, sl),
                "wvt": np.ascontiguousarray(np.asarray(wv, np.float32)[sl].T).astype(BF),
                "wot": np.ascontiguousarray(np.asarray(wo, np.float32)[:, sl].T).astype(BF),
                "bqk": bqk,
                "om": om,
            }
        )
    return in_maps


def out_bias(bv, wo, bo):
    """Host-side constant: bo + bv @ wo^T (softmax rows sum to 1)."""
    return (
        np.asarray(bo, np.float64)
        + np.asarray(bv, np.float64) @ np.asarray(wo, np.float64).T
    ).astype(np.float32)


_NC_CACHE = {}


def get_program():
    if "nc" not in _NC_CACHE:
        _NC_CACHE["nc"] = build_program()
    return _NC_CACHE["nc"]


def run_sharded(inputs, trace=False):
    nc = get_program()
    in_maps = shard_inputs(**inputs)
    res = run_bass_kernel_spmd(nc, in_maps, list(range(N_CORES)), trace=trace)
    bias = out_bias(inputs["bv"], inputs["wo"], inputs["bo"])
    full = np.empty((BATCH, SEQ, D_MODEL), np.float32)
    for b in range(BATCH):
        acc = res.results[b * N_GROUPS]["out"].astype(np.float32)
        for g in range(1, N_GROUPS):
            acc += res.results[b * N_GROUPS + g]["out"].astype(np.float32)
        full[b] = acc + bias
    return full, res


def kernel(**inputs):
    out, _ = run_sharded(inputs, trace=False)
    return out
